# revision 4
# baseline (speedup 1.0000x reference)
"""AnomalyAttention Trainium2 kernel (8 NeuronCores, SPMD data-parallel over batch).

Math (per b,h):
  series = softmax(causal_mask(Q K^T / 8))          = E / sum(E)
  prior  = rownorm(exp(-(l-s)^2 / (2 sigma'^2)))    (banded: |l-s|<=16 exact)
  out    = a*(E@V) + b*(G@V),  a = g/sum(E), b = (1-g)/sum(G)  per row.

v2 structure (per core: 2 batches x 8 heads; ACT/exp is the bottleneck):
  - scores TRANSPOSED (S^T = K Q^T) packed into ONE 3-bank PSUM tile
    [128, 1296] (sj0|sj1|sj3|sj2 = 512|384|128|256 cols, no matmul crosses
    a bank) -> ONE cross-bank exp per head (1280 cols) -> ET bf16 lhsT.
  - Gaussian prior via aligned "pair" scheme: pairs q cover l in
    [64q,64q+64), s-window [64q-32,64q+96).  Premul m*d^2 in [l,s] layout
    (per-partition m ptr on Pool), 4 PE transposes into a PSUM tile, ONE
    exp [128,512] PSUM->SBUF -> GT bf16; U2 = 8 aligned matmuls against
    shifted V tiles T_q.  No DVE copies of G, no extra exp columns.
  - row sums via ones-column matmuls into spare score-tile columns
    [1280:1288) (sum_g) and [1284... sum_e at [1284:1288)... see code.
  - normalization: DVE copies U (PSUM->SBUF), Pool tensor_scalar + DVE
    scalar_tensor_tensor produce out = a*U1 + b*U2.
  - V tiles loaded as fused multi-block SWDGE cast DMAs straight from HBM.
"""

import math
from contextlib import ExitStack

import ml_dtypes
import numpy as np

import concourse.bass as bass
import concourse.mybir as mybir
import concourse.tile as tile
from concourse import bacc
from concourse.bass_utils import run_bass_kernel_spmd

F32 = mybir.dt.float32
BF16 = mybir.dt.bfloat16
AF = mybir.ActivationFunctionType
OP = mybir.AluOpType

B, L, H, E = 16, 512, 8, 64
NCORES = 8
BPC = B // NCORES  # batches per core
PC = 128
NCH = 4
POISON = 1e12
LN3 = math.log(3.0)

# packed score-tile layout: sj -> (diag col, tail col, tail l-range)
# sj0 [0:512], sj1 [512:896], sj3 [896:1024], sj2 [1024:1280]
SJ_BASE = {0: 0, 1: 512, 3: 896, 2: 1024}
SG_COL = 1280  # sum_g cols [1280:1284), sum_e cols [1284:1288)
SE_COL = 1284

_CACHE = {}
LAST_RESULT = None


def _et_col(sj, li):
    """Packed ET column of l = 128*li for s-block sj (li >= sj)."""
    return SJ_BASE[sj] + 128 * (li - sj)


def _consts():
    ident = np.eye(PC, dtype=ml_dtypes.bfloat16)
    identf = np.eye(PC, dtype=np.float32)
    # mask for S^T diag block: -240 where l < s (strict lower triangle)
    mtri = np.tril(np.full((PC, PC), -240.0, dtype=np.float32), k=-1).astype(
        ml_dtypes.bfloat16
    )
    # d2pm[p, 128k + j] = ((p%64)+32-j)^2, poisoned outside |d|<=16 / seq edge
    p = np.arange(PC)[:, None]
    j = np.arange(PC)[None, :]
    d = (p % 64) + 32 - j
    base = np.where(np.abs(d) <= 16, (d * d).astype(np.float32), POISON)
    d2pm = np.zeros((PC, 4 * PC), np.float32)
    for k in range(4):
        blk = base.copy()
        q2 = 2 * k + p // 64  # pair index per partition
        s = 64 * q2 - 32 + j
        blk[(s < 0) | (s >= L)] = POISON
        d2pm[:, PC * k:PC * (k + 1)] = blk
    ones_col = np.ones((PC, 1), dtype=ml_dtypes.bfloat16)
    return ident, identf, mtri, d2pm, ones_col


def _build():
    if "nc" in _CACHE:
        return _CACHE["nc"]
    nc = bacc.Bacc()
    ident_np, identf_np, mtri_np, d2pm_np, ones_np = _consts()

    q_h = nc.dram_tensor("queries", [BPC, L, H, E], F32, kind="ExternalInput")
    k_h = nc.dram_tensor("keys", [BPC, L, H, E], F32, kind="ExternalInput")
    v_h = nc.dram_tensor("values", [BPC, L, H, E], F32, kind="ExternalInput")
    sig_h = nc.dram_tensor("sigma", [BPC, L, H], F32, kind="ExternalInput")
    hgl_h = nc.dram_tensor("hgl", [1, H], F32, kind="ExternalInput")
    out_h = nc.dram_tensor("out", [BPC, L, H, E], F32, kind="ExternalOutput")

    ident_d = nc.inline_tensor(ident_np, name="identc")
    identf_d = nc.inline_tensor(identf_np, name="identfc")
    mtri_d = nc.inline_tensor(mtri_np, name="mtric")
    d2pm_d = nc.inline_tensor(d2pm_np, name="d2pmc")
    ones_d = nc.inline_tensor(ones_np, name="onesc")

    with ExitStack() as ctx:
        tc = ctx.enter_context(tile.TileContext(nc))
        const = ctx.enter_context(tc.tile_pool(name="const", bufs=1))
        qkT = ctx.enter_context(tc.tile_pool(name="qkT", bufs=2))
        vpool = ctx.enter_context(tc.tile_pool(name="vpool", bufs=2))
        spool = ctx.enter_context(tc.tile_pool(name="spool", bufs=1))
        etpool = ctx.enter_context(tc.tile_pool(name="etpool", bufs=2))
        pmpool = ctx.enter_context(tc.tile_pool(name="pmpool", bufs=2))
        gtpool = ctx.enter_context(tc.tile_pool(name="gtpool", bufs=2))
        ucpool = ctx.enter_context(tc.tile_pool(name="ucpool", bufs=2))
        small = ctx.enter_context(tc.tile_pool(name="small", bufs=4))
        outp = ctx.enter_context(tc.tile_pool(name="outp", bufs=2))
        ps_s = ctx.enter_context(tc.tile_pool(name="ps_s", bufs=2, space="PSUM"))
        ps_u = ctx.enter_context(tc.tile_pool(name="ps_u", bufs=1, space="PSUM"))
        ps_g = ctx.enter_context(tc.tile_pool(name="ps_g", bufs=1, space="PSUM"))
        dram = ctx.enter_context(tc.tile_pool(name="dram", bufs=2, space="DRAM"))

        # ---- constants ----
        ident = const.tile([PC, PC], BF16, tag="ident")
        nc.sync.dma_start(ident, ident_d[:, :])
        identf = const.tile([PC, PC], F32, tag="identf")
        nc.sync.dma_start(identf, identf_d[:, :])
        mtri = const.tile([PC, PC], BF16, tag="mtri")
        nc.sync.dma_start(mtri, mtri_d[:, :])
        d2pm = const.tile([PC, 4 * PC], F32, tag="d2pm")
        nc.sync.dma_start(d2pm, d2pm_d[:, :])
        ones_col = const.tile([PC, 1], BF16, tag="ones")
        nc.sync.dma_start(ones_col, ones_d[:, :])

        # ---- sigma -> m for both batches: one fused [128, 64] chain ----
        SW = BPC * NCH * H  # 64: col = b*32 + c*8 + h ; partition p -> l=128c+p
        sraw = spool.tile([PC, SW], F32, tag="sraw")
        nc.sync.dma_start(
            sraw[:, :].rearrange("p (b c h) -> p b c h", b=BPC, c=NCH),
            sig_h[:, :, :].rearrange("b (c p) h -> p b c h", p=PC),
        )
        e5 = spool.tile([PC, SW], F32, tag="e5")
        nc.scalar.activation(e5, sraw, AF.Exp, scale=-5.0)
        p1 = spool.tile([PC, SW], F32, tag="p1")
        nc.vector.tensor_scalar_add(p1, e5, 1.0)
        sg = spool.tile([PC, SW], F32, tag="sg")
        nc.vector.reciprocal(sg, p1)
        sg2 = spool.tile([PC, SW], F32, tag="sg2")
        nc.vector.tensor_scalar_add(sg2, sg, 1e-5)
        p3 = spool.tile([PC, SW], F32, tag="p3")
        nc.scalar.activation(p3, sg2, AF.Exp, scale=LN3)
        sm1 = spool.tile([PC, SW], F32, tag="sm1")
        nc.vector.tensor_scalar_add(sm1, p3, -1.0)
        s2 = spool.tile([PC, SW], F32, tag="s2")
        nc.vector.tensor_tensor(s2, sm1, sm1, OP.mult)
        r2 = spool.tile([PC, SW], F32, tag="r2")
        nc.vector.reciprocal(r2, s2)
        m_all = spool.tile([PC, SW], F32, tag="m_all")
        nc.vector.tensor_scalar_mul(m_all, r2, -0.5)

        # ---- gates (sigmoid -> per-partition broadcast) ----
        hgl_sb = const.tile([1, H], F32, tag="hgl")
        nc.sync.dma_start(hgl_sb, hgl_h[:, :])
        ge = const.tile([1, H], F32, tag="ge")
        nc.scalar.activation(ge, hgl_sb, AF.Exp, scale=-1.0)
        gp = const.tile([1, H], F32, tag="gp")
        nc.vector.tensor_scalar_add(gp, ge, 1.0)
        gate = const.tile([1, H], F32, tag="gate")
        nc.vector.reciprocal(gate, gp)
        onesf = const.tile([1, PC], F32, tag="onesf")
        nc.vector.memset(onesf, 1.0)
        gb_ps = ps_u.tile([PC, 512], F32, tag="U")
        nc.tensor.matmul(gb_ps[:, 0:H], onesf, gate, start=True, stop=True)
        gates_b = const.tile([PC, H], F32, tag="gatesb")
        nc.vector.tensor_copy(gates_b, gb_ps[:, 0:H])
        omg_b = const.tile([PC, H], F32, tag="omgb")
        nc.vector.tensor_scalar(omg_b, gates_b, -1.0, 1.0, OP.mult, OP.add)

        for bi in range(BPC):
            # ---- Q/K: SWDGE cast f32->bf16 to DRAM scratch (one DMA each) ----
            qscr = dram.tile([L, H * E], BF16, tag="qscr")
            kscr = dram.tile([L, H * E], BF16, tag="kscr")
            nc.gpsimd.dma_start(qscr, q_h[bi, :, :, :])
            nc.gpsimd.dma_start(kscr, k_h[bi, :, :, :])
            QT = []
            KT = []
            for to in range(4):
                qt = qkT.tile([PC, L], BF16, tag=f"qT{to}")
                kt = qkT.tile([PC, L], BF16, tag=f"kT{to}")
                nc.sync.dma_start_transpose(qt, qscr[:, to * PC:(to + 1) * PC])
                nc.sync.dma_start_transpose(kt, kscr[:, to * PC:(to + 1) * PC])
                QT.append(qt)
                KT.append(kt)

            # ---- V tiles: natural chunks + shifted pair family ----
            Vn = vpool.tile([PC, NCH, H * E], BF16, tag="vn")
            nc.gpsimd.dma_start(
                Vn, v_h[bi, :, :, :].rearrange("(c p) h e -> p c (h e)", p=PC)
            )
            Tt = vpool.tile([PC, 8, H * E], BF16, tag="tt")
            # pair tile q holds V rows [64q-32, 64q+96); windows overlap so
            # each q is its own load; q=0/7 are partial + memset zero edges.
            _t_src = v_h[bi, :, :, :].rearrange("r h e -> r (h e)")
            for qq in range(1, 7):
                nc.gpsimd.dma_start(
                    Tt[:, qq:qq + 1, :], _t_src[64 * qq - 32:64 * qq + 96, :]
                )
            nc.gpsimd.dma_start(Tt[32:PC, 0:1, :], _t_src[0:96, :])
            nc.gpsimd.dma_start(Tt[0:96, 7:8, :], _t_src[416:512, :])
            nc.vector.memset(Tt[0:32, 0:1, :], 0.0)
            nc.vector.memset(Tt[96:PC, 7:8, :], 0.0)

            mo = bi * 32
            oslab = []
            for li in range(NCH):
                ot = outp.tile([PC, H * E], F32, tag=f"o{li}")
                oslab.append(ot)

            for h in range(H):
                th = h // 2
                po = 64 * (h % 2)

                # ---- packed scores + ONE exp ----
                S = ps_s.tile([PC, 1296], F32, tag="S", padded_shape=[PC, 1536])
                for sj in range(4):
                    cb = SJ_BASE[sj]
                    lh = KT[th][po:po + 64, sj * PC:(sj + 1) * PC]
                    nc.tensor.matmul(
                        S[:, cb:cb + PC], lh,
                        QT[th][po:po + 64, sj * PC:(sj + 1) * PC],
                        start=True, stop=False,
                    )
                    nc.tensor.matmul(
                        S[:, cb:cb + PC], ident, mtri, start=False, stop=True
                    )
                    nw = L - (sj + 1) * PC
                    if nw > 0:
                        nc.tensor.matmul(
                            S[:, cb + PC:cb + PC + nw], lh,
                            QT[th][po:po + 64, (sj + 1) * PC:L],
                            start=True, stop=True,
                        )
                ET = etpool.tile([PC, 1280], BF16, tag="ET")
                nc.scalar.activation(ET, S[:, 0:1280], AF.Exp, scale=0.125)

                # ---- G path: premul [l,s] -> PE transpose -> exp ----
                PM = pmpool.tile([PC, 512], F32, tag="PM")
                for k in range(NCH):
                    nc.gpsimd.tensor_scalar_mul(
                        PM[:, PC * k:PC * (k + 1)],
                        d2pm[:, PC * k:PC * (k + 1)],
                        m_all[:, mo + k * H + h:mo + k * H + h + 1],
                    )
                GTp = ps_g.tile([PC, 512], F32, tag="GT")
                for k in range(NCH):
                    nc.tensor.transpose(
                        GTp[:, PC * k:PC * (k + 1)], PM[:, PC * k:PC * (k + 1)],
                        identf,
                    )
                GT = gtpool.tile([PC, 512], BF16, tag="GTs")
                nc.scalar.activation(GT, GTp[:, :], AF.Exp)

                # ---- U1 + sum_e ----
                U = ps_u.tile([PC, 512], F32, tag="U")
                for li in range(NCH):
                    for sj in range(li + 1):
                        ec = _et_col(sj, li)
                        nc.tensor.matmul(
                            U[:, 64 * li:64 * li + 64],
                            ET[:, ec:ec + PC],
                            Vn[:, sj:sj + 1, 64 * h:64 * h + 64],
                            start=(sj == 0), stop=(sj == li),
                        )
                        nc.tensor.matmul(
                            S[:, SE_COL + li:SE_COL + li + 1],
                            ET[:, ec:ec + PC], ones_col,
                            start=(sj == 0), stop=(sj == li),
                        )

                # ---- U2 + sum_g (aligned pair matmuls) ----
                for q in range(8):
                    k = q // 2
                    pb = 64 * (q & 1)
                    nc.tensor.matmul(
                        U[pb:pb + 64, 256 + 64 * k:256 + 64 * k + 64],
                        GT[:, 64 * q:64 * q + 64],
                        Tt[:, q:q + 1, 64 * h:64 * h + 64],
                        start=True, stop=True,
                    )
                    nc.tensor.matmul(
                        S[pb:pb + 64, SG_COL + k:SG_COL + k + 1],
                        GT[:, 64 * q:64 * q + 64], ones_col,
                        start=True, stop=True,
                    )

                # ---- normalization ----
                re = small.tile([PC, NCH], F32, tag="re")
                nc.vector.reciprocal(re, S[:, SE_COL:SE_COL + 4])
                rg = small.tile([PC, NCH], F32, tag="rg")
                nc.vector.reciprocal(rg, S[:, SG_COL:SG_COL + 4])
                av = small.tile([PC, NCH], F32, tag="av")
                nc.gpsimd.tensor_scalar_mul(av, re, gates_b[:, h:h + 1])
                bv = small.tile([PC, NCH], F32, tag="bv")
                nc.gpsimd.tensor_scalar_mul(bv, rg, omg_b[:, h:h + 1])
                Uc = ucpool.tile([PC, 512], F32, tag="Uc")
                nc.vector.tensor_copy(Uc, U[:, :])
                for li in range(NCH):
                    t2 = small.tile([PC, 64], F32, tag="t2")
                    nc.gpsimd.tensor_scalar_mul(
                        t2, Uc[:, 256 + 64 * li:256 + 64 * li + 64],
                        bv[:, li:li + 1],
                    )
                    nc.vector.scalar_tensor_tensor(
                        oslab[li][:, 64 * h:64 * h + 64],
                        Uc[:, 64 * li:64 * li + 64],
                        av[:, li:li + 1], t2, OP.mult, OP.add,
                    )

                if h == 3:
                    for li in range(NCH):
                        nc.sync.dma_start(
                            out_h[bi, li * PC:(li + 1) * PC, 0:4, :],
                            oslab[li][:, 0:256],
                        )
            for li in range(NCH):
                nc.sync.dma_start(
                    out_h[bi, li * PC:(li + 1) * PC, 4:8, :],
                    oslab[li][:, 256:512],
                )

    nc.compile()
    _CACHE["nc"] = nc
    return nc


def kernel(**inputs):
    global LAST_RESULT
    nc = _build()
    q = np.ascontiguousarray(inputs["queries"], dtype=np.float32)
    k = np.ascontiguousarray(inputs["keys"], dtype=np.float32)
    v = np.ascontiguousarray(inputs["values"], dtype=np.float32)
    sg = np.ascontiguousarray(inputs["sigma"], dtype=np.float32)
    hgl = np.ascontiguousarray(
        inputs["head_gate_logit"], dtype=np.float32
    ).reshape(1, H)

    in_maps = []
    for c in range(NCORES):
        b0 = BPC * c
        in_maps.append({
            "queries": q[b0:b0 + BPC],
            "keys": k[b0:b0 + BPC],
            "values": v[b0:b0 + BPC],
            "sigma": sg[b0:b0 + BPC],
            "hgl": hgl,
        })
    res = run_bass_kernel_spmd(nc, in_maps, core_ids=list(range(NCORES)))
    LAST_RESULT = res
    out = np.concatenate([r["out"] for r in res.results], axis=0)
    return out.astype(np.float32)
